# revision 1
# baseline (speedup 1.0000x reference)
"""CTRNN cell (6 Euler unfolds) on 8 Trainium2 NeuronCores.

Math (per unfold, 6x):
    f     = tanh([x, s] @ W + b)
    s_new = s + 0.1 * (-s + f)  = 0.9*s + 0.1*f

Strategy:
  - Data-parallel over batch: B=8192 -> 1024 rows/core, no cross-core
    communication. Host does the cheap numpy transposes/packing.
  - Everything kept TRANSPOSED on-chip (feature dim on SBUF partitions,
    batch on the free dim) so state feeds the tensor engine as the moving
    operand and W slices are directly the stationary lhsT.
  - pre = x @ W_top is computed once. Per-unfold matmuls run in *delta*
    form: one PSUM accumulator per output m-tile holds pre + s_k @ W_bot
    across all unfolds, updated with psum += (f_k - s_k) @ (0.1*W_bot).
    This is the 7-logical-matmul FLOP floor and PSUM never restarts.
  - All matmuls in float32r (fp32-precision inputs at bf16 rate for free
    dim >= 512). Inputs are DMA'd directly into f32r-typed tiles (walrus
    accepts a DMACopy with f32r output dtype as the required rounding
    producer; verified bit-identical to an explicit DVE cast on HW).
    The state is kept in plain f32 (so the per-unfold update never rounds
    it to f32r); a one-time f32r copy feeds the init matmul.
  - x, s and W all arrive host-packed as (128, k*1024) layouts so every
    DMA has >=4KB-contiguous per-partition runs - the DMA queues are
    descriptor-rate bound (4KB is the max HW descriptor; 2KB rows halve
    bandwidth). Input is spread over the SWDGE path and both HWDGE rings.
  - PSUM per m-tile is one (128,1024) span (2 banks); matmuls write
    512-wide halves, tanh reads the full span (amortizes ACT op overhead).
  - bias is folded into the tanh activation's per-partition bias operand.
  - A junk-matmul warm-up keeps the PE activity monitor from throttling
    the clock to 1.2 GHz during the input-load phase.
  - Steady state is vector-engine paced (2 fused scalar_tensor_tensor
    passes per m-tile per unfold: tmp = f - s, then s += 0.1*tmp).
"""

import numpy as np

UNFOLDS = 6
DT = 0.1
B, D, N = 8192, 512, 512
NCORES = 8
BC = B // NCORES          # batch rows per core
CHUNK = 512               # matmul moving-operand free dim (PSUM bank)
NCHUNKS = BC // CHUNK     # 2
P = 128
KT_X = D // P             # k-tiles of W_top
KT_S = N // P             # k-tiles of W_bot
MT = N // P               # m-tiles of the output dim

_compiled_nc = None


def _build_nc():
    import concourse.bass as bass  # noqa: F401
    import concourse.bacc as bacc
    import concourse.tile as tile
    from concourse import mybir

    f32 = mybir.dt.float32
    f32r = mybir.dt.float32r
    bf16 = mybir.dt.bfloat16
    MULT = mybir.AluOpType.mult
    ADD = mybir.AluOpType.add
    TANH = mybir.ActivationFunctionType.Tanh

    nc = bacc.Bacc("TRN2", target_bir_lowering=False, debug=False)

    xP = nc.dram_tensor("xP", [P, D * BC // P], f32r, kind="ExternalInput").ap()
    sP = nc.dram_tensor("sP", [P, N * BC // P], f32, kind="ExternalInput").ap()
    Wp_d = nc.dram_tensor("Wp", [P, (D + N) * N // P], f32r,
                          kind="ExternalInput").ap()
    bias = nc.dram_tensor("bias", [N], f32, kind="ExternalInput").ap()
    outT = nc.dram_tensor("outT", [N, BC], f32, kind="ExternalOutput").ap()

    with tile.TileContext(nc) as tc:
        with (
            tc.tile_pool(name="weights", bufs=1) as wpool,
            tc.tile_pool(name="dmain", bufs=3) as dmain,
            tc.tile_pool(name="data", bufs=1) as data,
            tc.tile_pool(name="tmp", bufs=2) as tmpp,
            tc.tile_pool(name="fpool", bufs=3) as fpool,
            tc.tile_pool(name="psum", bufs=1, space="PSUM") as psump,
        ):
            # ---- input DMAs (all f32r-direct, no rounding casts) -----------
            # walrus accepts DMACopy with f32r output as the rounding
            # producer for f32r matmuls (verified on HW: identical result to
            # an explicit DVE cast). W arrives host-packed as (128, 4096) so
            # every DMA has 4KB-contiguous per-partition runs (the DMA queues
            # are descriptor-rate-bound: 2KB rows halve the bandwidth).
            # Load is balanced across SWDGE (~200 GB/s) and the two HWDGE
            # rings; everything lands by ~HBM-bound time.
            # HAM warm-up part 1: memset a junk tile first thing on the
            # gpsimd queue (before its DMA issues) so the warm-up matmuls can
            # start immediately.
            junk = wpool.tile([P, N], bf16, tag="junk", name="junk")
            nc.gpsimd.memset(junk[:], 0)

            NPAIR = (KT_X + KT_S) // 2
            # x and s are host-packed like W so every DMA has >=8KB-
            # contiguous per-partition runs (DMA queues are descriptor-rate
            # bound). x on the sync ring, s on SWDGE, W on the scalar ring;
            # two half-DMAs per tensor so the first k-tiles land early.
            wp = []
            for q in range(NPAIR):
                wd = wpool.tile([P, 2 * N], f32r, tag=f"wp{q}", name=f"wp{q}")
                eng = nc.gpsimd if q == NPAIR - 1 else nc.scalar
                eng.dma_start(wd[:], Wp_d[:, q * 2 * N:(q + 1) * 2 * N])
                wp.append(wd)

            HALF = D * BC // P // 2
            x_mega = data.tile([P, 2 * HALF], f32r, tag="xm", name="x_mega")
            nc.sync.dma_start(x_mega[:, 0:HALF], xP[:, 0:HALF])
            nc.sync.dma_start(x_mega[:, HALF:2 * HALF], xP[:, HALF:2 * HALF])
            x_sb = [x_mega[:, j * BC:(j + 1) * BC] for j in range(KT_X)]

            s_mega = data.tile([P, 2 * HALF], f32, tag="sm", name="s_mega")
            nc.gpsimd.dma_start(s_mega[:, 0:HALF], sP[:, 0:HALF])
            nc.gpsimd.dma_start(s_mega[:, HALF:2 * HALF], sP[:, HALF:2 * HALF])
            s_sb = [s_mega[:, j * BC:(j + 1) * BC] for j in range(KT_S)]

            s_r = []
            for j in range(KT_S):
                tr = data.tile([P, BC], f32r, tag=f"sr{j}", name=f"sr{j}")
                nc.vector.tensor_copy(tr[:], s_sb[j])
                s_r.append(tr)
            bias_sb = wpool.tile([P, MT], f32, tag="bias", name="bias_sb")
            nc.gpsimd.dma_start(bias_sb[:], bias.rearrange("(m p) -> p m", p=P))

            # the only casts left: 0.1*W_bot in bf16 for the delta matmuls
            wbp01h = []
            for q in range(NPAIR // 2):
                w = wpool.tile([P, 2 * N], f32r, tag=f"wbph{q}",
                               name=f"wbp01h_{q}")
                nc.scalar.mul(w[:], wp[NPAIR // 2 + q][:], DT)
                wbp01h.append(w)

            def pair_slices(pairs):
                out = []
                for w in pairs:
                    out.append(w[:, 0:N])
                    out.append(w[:, N:2 * N])
                return out

            wt = pair_slices(wp[:NPAIR // 2])       # W_top f32r k-slices
            wbot = pair_slices(wp[NPAIR // 2:])     # W_bot f32r k-slices
            wb01h = pair_slices(wbp01h)             # 0.1*W_bot bf16 k-slices

            # ---- persistent PSUM accumulators: pre + s_k @ W_bot ----------
            # one (128, 1024) span per m-tile = 2 banks; matmuls address
            # 512-wide halves, ACT reads the whole span.
            ps = [psump.tile([P, BC], f32, tag=f"ps{m}", name=f"ps{m}")
                  for m in range(MT)]

            # HAM warm-up part 2: junk matmuls keep the PE busy while the
            # inputs stream in, so real matmuls run at 2.4 GHz from the start
            # (the activity monitor needs ~3.4us of sustained work to
            # unthrottle). Overwritten by the first start=True matmul per bank.
            for r in range(20):
                nc.tensor.matmul(
                    ps[r % MT][:, 0:CHUNK],
                    lhsT=junk[:, 0:P], rhs=junk[:, 0:CHUNK],
                    start=True, stop=True, skip_group_check=True,
                )

            def mm_round(weights, rhs_tiles, start, stop, m_outer=False):
                nkt = len(rhs_tiles)
                order = (
                    [(j, m) for m in range(MT) for j in range(nkt)]
                    if m_outer else
                    [(j, m) for j in range(nkt) for m in range(MT)]
                )
                for j, m in order:
                    for c in range(NCHUNKS):
                        nc.tensor.matmul(
                            ps[m][:, c * CHUNK:(c + 1) * CHUNK],
                            lhsT=weights[j][:, m * P:(m + 1) * P],
                            rhs=rhs_tiles[j][:, c * CHUNK:(c + 1) * CHUNK],
                            start=(start and j == 0),
                            stop=(stop and j == nkt - 1),
                            skip_group_check=True,
                        )

            # init: psum = x @ W_top + s0 @ W_bot
            mm_round(wt, x_sb, start=True, stop=False)
            mm_round(wb01h, s_r, start=False, stop=False)

            # ---- unfolds ---------------------------------------------------
            # state kept scaled: v = 10*s. tmp = f - 0.1*v (== f - s) feeds
            # the delta matmuls; the state update becomes the plain add
            # v += tmp, which runs on the otherwise-idle GpSimd engine and
            # frees half of the vector-engine work per unfold.
            for k in range(UNFOLDS):
                last = k == UNFOLDS - 1
                tmp_t = [tmpp.tile([P, BC], f32r, tag=f"tmp{j}",
                                   name=f"tmp{k}_{j}")
                         for j in range(MT)]
                f_t = [fpool.tile([P, BC], f32, tag=f"f{m}", name=f"f{k}_{m}",
                                  bufs=2)
                       for m in range(MT)]
                if not last:
                    for m in range(MT):
                        # f = tanh(psum + bias), full (128,1024) span
                        nc.scalar.activation(
                            f_t[m][:], ps[m][:], TANH,
                            bias=bias_sb[:, m:m + 1], scale=1.0,
                        )
                        # tmp = f - 0.1*v (f32r out, feeds the delta matmuls)
                        nc.vector.scalar_tensor_tensor(
                            tmp_t[m][:], s_sb[m], -DT, f_t[m][:],
                            op0=MULT, op1=ADD,
                        )
                    # psum += tmp @ (0.1*W_bot)
                    mm_round(wb01h, tmp_t, start=False,
                             stop=(k == UNFOLDS - 2))
                    # v += tmp (plain add, lazy: emitted after the matmuls)
                    for m in range(MT):
                        nc.vector.tensor_tensor(
                            s_sb[m], s_sb[m], tmp_t[m][:], ADD,
                        )
                else:
                    # final unfold: s_out = 0.1*(0.9*v + f) = 0.9*s + 0.1*f;
                    # the descale is a fast single-src tensor_scalar
                    for m in range(MT):
                        nc.scalar.activation(
                            f_t[m][:], ps[m][:], TANH,
                            bias=bias_sb[:, m:m + 1], scale=1.0,
                        )
                        nc.vector.scalar_tensor_tensor(
                            f_t[m][:], s_sb[m], 0.9, f_t[m][:],
                            op0=MULT, op1=ADD,
                        )
                        nc.vector.tensor_scalar_mul(
                            s_sb[m].bitcast(f32), f_t[m][:], DT)
                        out_eng = (nc.sync, nc.scalar, nc.gpsimd,
                                   nc.sync)[m]
                        out_eng.dma_start(outT[m * P:(m + 1) * P, :],
                                          s_sb[m].bitcast(f32))

    nc.compile()
    return nc


def _get_nc():
    global _compiled_nc
    if _compiled_nc is None:
        _compiled_nc = _build_nc()
    return _compiled_nc


def make_in_maps(x, s, W, b):
    """Shard + pack host-side: x/s/W packed to (128, k*1024) with k-tiles
    side by side so per-partition runs are >=8KB contiguous."""
    xT = np.ascontiguousarray(x.T)   # (D, B)
    sTf = np.ascontiguousarray(s.T)  # (N, B)
    Wp = np.ascontiguousarray(
        W.reshape(4, 2, P, N).transpose(2, 0, 1, 3).reshape(P, -1))
    in_maps = []
    for c in range(NCORES):
        sl = slice(c * BC, (c + 1) * BC)
        xs = xT[:, sl].reshape(KT_X, P, BC).transpose(1, 0, 2).reshape(P, -1)
        ss = (10.0 * sTf[:, sl]).reshape(KT_S, P, BC).transpose(1, 0, 2).reshape(P, -1)
        in_maps.append({
            "xP": np.ascontiguousarray(xs),
            "sP": np.ascontiguousarray(ss),
            "Wp": Wp,
            "bias": b,
        })
    return in_maps


def kernel(**inputs):
    from concourse.bass_utils import run_bass_kernel_spmd

    x = np.asarray(inputs["inputs"], dtype=np.float32)
    s = np.asarray(inputs["state"], dtype=np.float32)
    W = np.ascontiguousarray(np.asarray(inputs["W"], dtype=np.float32))
    b = np.ascontiguousarray(np.asarray(inputs["bias"], dtype=np.float32))

    in_maps = make_in_maps(x, s, W, b)
    nc = _get_nc()
    res = run_bass_kernel_spmd(nc, in_maps, list(range(NCORES))).results
    outT = np.concatenate([res[c]["outT"] for c in range(NCORES)], axis=1)
    out = np.ascontiguousarray(outT.T).astype(np.float32)
    return (out, out)

